# revision 1
# baseline (speedup 1.0000x reference)
"""GCN classifier Trainium2 kernel (8-core SPMD, Bass/Tile).

Model (reference):
    h1 = relu(gcnconv(x, W1, b1));  h2 = gcnconv(h1, W2, b2);  out = mean-pool(h2, batch)

Distribution strategy (no cross-core communication):
  * Nodes sharded contiguously across 8 cores (6250 each); x replicated (fp16).
  * Layer-1 aggregation per dst shard. Edge rows are fetched with BULK
    dma_gather (mlp gpsimd library): one instruction gathers ~7k rows for a
    7-block group, split into lo/hi source halves because gather indices are
    int16. Per 128-edge tile, the selection matrix
    S_T[e,d] = w_e * (dst_e == d) (w_e = dinv[src]*dinv[dst]) is built in one
    fused DVE tensor_scalar; scatter = fp16 matmul accumulating into f32 PSUM.
  * h1^T and z2 = h1 @ W2 stay on-chip (PSUM/SBUF), never round-trip HBM.
  * Layer 2 + mean-pool collapse algebraically:
        pool_sums[g,f] = sum_e w_e * z2[src_e, f] * [batch[dst_e] == g]
                       = sum_n C[g,n] * z2[n,f]
    with C built host-side from indices/weights only -> dense matmuls, zero
    communication. Host sums 8 partials, divides by counts, adds b2.

Numerics: fp16 operand quantization (~5e-4) with exact f32 PSUM accumulation.
(True f32 matmuls on TRN2 PE are bf16-class 2-pass approximations anyway.)
"""

import math
import numpy as np

N_NODES = 50000
N_EDGES = 600000
N_GRAPHS = 64
IN_DIM = 128
HID_DIM = 128
OUT_DIM = 64
N_CORES = 8
P = 128
GB = 7              # blocks per gather group
HALF = N_NODES // 2


# ---------------------------------------------------------------- host prep
def _host_prep(x, edge_index, batch):
    n = x.shape[0]
    half = n // 2
    shard = n // N_CORES                    # 6250
    n_blocks = math.ceil(shard / P)         # 49
    shard_pad = n_blocks * P                # 6272
    n_groups = math.ceil(n_blocks / GB)     # 7

    src = np.asarray(edge_index[0], dtype=np.int64)
    dst = np.asarray(edge_index[1], dtype=np.int64)
    batch = np.asarray(batch, dtype=np.int64)

    deg = np.bincount(dst, minlength=n).astype(np.float32) + np.float32(1.0)
    dinv = (np.float32(1.0) / np.sqrt(deg)).astype(np.float32)

    loops = np.arange(n, dtype=np.int64)
    SRC = np.concatenate([src, loops])
    DST = np.concatenate([dst, loops])
    W = (dinv[SRC] * dinv[DST]).astype(np.float32)
    E = SRC.shape[0]

    # ---- bucket edges by (core, block, src-half) of DST/SRC
    core_of = DST // shard
    blk_of = (DST % shard) // P
    dstl = (DST % shard) % P
    half_of = SRC // half

    n_buckets_per_core = n_blocks * 2
    bucket = (core_of * n_blocks + blk_of) * 2 + half_of
    order = np.argsort(bucket, kind="stable")
    bucket_s = bucket[order]
    counts = np.bincount(bucket_s, minlength=N_CORES * n_buckets_per_core)
    T2 = int(math.ceil(counts.max() / P))   # uniform tiles per (block, half)

    cum = np.zeros(N_CORES * n_buckets_per_core + 1, dtype=np.int64)
    np.cumsum(counts, out=cum[1:])
    pos = np.arange(E) - cum[bucket_s]

    # tile-column order: col(g, half, b_in_g, j) = ((g*2 + half)*GB + b)*T2 + j
    ntiles = n_blocks * 2 * T2              # columns per core
    e_core = core_of[order]
    e_blk = blk_of[order]
    e_half = half_of[order]
    e_g = e_blk // GB
    e_b = e_blk % GB
    col = ((e_g * 2 + e_half) * GB + e_b) * T2 + pos // P
    row = pos % P

    w_cols = np.zeros((N_CORES, P, ntiles), dtype=np.float32)
    dstl_cols = np.zeros((N_CORES, P, ntiles), dtype=np.float32)
    w_cols[e_core, row, col] = W[order]
    dstl_cols[e_core, row, col] = dstl[order].astype(np.float32)

    # gather indices: flat slot i = col*128 + row; idx layout [16, i//16] tiled
    flat_idx = np.zeros((N_CORES, ntiles * P), dtype=np.int16)
    slot = col * P + row
    flat_idx[e_core, slot] = (SRC[order] % half).astype(np.int16)
    nic = ntiles * P // 16                  # int16 idx columns per core
    gidx16 = flat_idx.reshape(N_CORES, nic, 16).transpose(0, 2, 1)  # [C,16,nic]
    gidx = np.ascontiguousarray(
        np.tile(gidx16, (1, 8, 1)))         # replicate to [C, 128, nic]

    # ---- layer-2 dense matrix C[g, n] = sum_{e: src=n} w_e * [batch[dst_e]=g]
    g_of = batch[DST]
    idx = ((SRC // shard) * N_GRAPHS + g_of) * shard + (SRC % shard)
    C = np.bincount(idx, weights=W.astype(np.float64),
                    minlength=N_CORES * N_GRAPHS * shard)
    C = C.reshape(N_CORES, N_GRAPHS, shard).astype(np.float32)

    Cp = np.zeros((N_CORES, N_GRAPHS, shard_pad), dtype=np.float32)
    Cp[:, :, :shard] = C
    CT_cols = Cp.reshape(N_CORES, N_GRAPHS, n_blocks, P).transpose(0, 3, 2, 1)
    CT_cols = np.ascontiguousarray(
        CT_cols.reshape(N_CORES, P, n_blocks * N_GRAPHS)).astype(np.float16)

    graph_counts = np.bincount(batch, minlength=N_GRAPHS).astype(np.float32)

    return dict(T2=T2, n_blocks=n_blocks, n_groups=n_groups, shard=shard,
                w_cols=w_cols, dstl_cols=dstl_cols, gidx=gidx,
                CT_cols=CT_cols, graph_counts=graph_counts)


# ---------------------------------------------------------------- bass program
_PROGRAM_CACHE = {}


def _build_program(T2, n_blocks, n_nodes, repeat=1):
    import concourse.bacc as bacc
    import concourse.tile as tile
    from concourse import mybir

    f32, i32 = mybir.dt.float32, mybir.dt.int32
    f16, i16 = mybir.dt.float16, mybir.dt.int16
    AF = mybir.ActivationFunctionType

    half = n_nodes // 2
    n_groups = math.ceil(n_blocks / GB)
    ntiles = n_blocks * 2 * T2
    nic = ntiles * P // 16
    gidx_per_gather = GB * T2 * P           # num_idxs per dma_gather
    gcols = gidx_per_gather // 16           # int16 cols per gather slice

    nc = bacc.Bacc("TRN2", target_bir_lowering=False, debug=False,
                   num_devices=N_CORES)
    x16_d = nc.dram_tensor("x16", [n_nodes, IN_DIM], f16, kind="ExternalInput")
    w1_d = nc.dram_tensor("w1", [IN_DIM, HID_DIM], f16, kind="ExternalInput")
    w2_d = nc.dram_tensor("w2", [HID_DIM, OUT_DIM], f16, kind="ExternalInput")
    b1_d = nc.dram_tensor("b1", [HID_DIM, 1], f32, kind="ExternalInput")
    iota_d = nc.dram_tensor("iota16", [P, P], f16, kind="ExternalInput")
    ident_d = nc.dram_tensor("ident", [P, P], f32, kind="ExternalInput")
    gidx_d = nc.dram_tensor("gidx", [P, nic], i16, kind="ExternalInput")
    wc_d = nc.dram_tensor("w_cols", [P, ntiles], f32, kind="ExternalInput")
    dstc_d = nc.dram_tensor("dstl_cols", [P, ntiles], f32, kind="ExternalInput")
    ctc_d = nc.dram_tensor("ct_cols", [P, n_blocks * N_GRAPHS], f16,
                           kind="ExternalInput")
    pool_d = nc.dram_tensor("pool_out", [N_GRAPHS, OUT_DIM], f32,
                            kind="ExternalOutput")

    with tile.TileContext(nc) as tc:
        with (
            tc.tile_pool(name="const", bufs=1) as cp,
            tc.tile_pool(name="work", bufs=4) as wp,
            tc.tile_pool(name="gat", bufs=2) as gp,
            tc.tile_pool(name="ps_out1", bufs=2, space="PSUM") as ps1,
            tc.tile_pool(name="ps_misc", bufs=1, space="PSUM") as ps2,
            tc.tile_pool(name="ps_pool", bufs=1, space="PSUM") as psp,
        ):
            iota16 = cp.tile([P, P], f16)
            nc.sync.dma_start(out=iota16[:], in_=iota_d[:])
            ident = cp.tile([P, P], f32)
            nc.sync.dma_start(out=ident[:], in_=ident_d[:])
            w1_t = cp.tile([IN_DIM, HID_DIM], f16)
            nc.sync.dma_start(out=w1_t[:], in_=w1_d[:])
            w2_t = cp.tile([HID_DIM, OUT_DIM], f16)
            nc.sync.dma_start(out=w2_t[:], in_=w2_d[:])
            b1_t = cp.tile([HID_DIM, 1], f32)
            nc.sync.dma_start(out=b1_t[:], in_=b1_d[:])
            gidx = cp.tile([P, nic], i16)
            nc.sync.dma_start(out=gidx[:], in_=gidx_d[:])
            wc = cp.tile([P, ntiles], f32)
            nc.sync.dma_start(out=wc[:], in_=wc_d[:])
            dstc = cp.tile([P, ntiles], f32)
            nc.sync.dma_start(out=dstc[:], in_=dstc_d[:])
            ctc = cp.tile([P, n_blocks * N_GRAPHS], f16)
            nc.sync.dma_start(out=ctc[:], in_=ctc_d[:])

            x_lo = x16_d[0:half, :]
            x_hi = x16_d[half:n_nodes, :]

            for _rep in range(repeat):
                pool_ps = psp.tile([N_GRAPHS, OUT_DIM], f32, space="PSUM",
                                   tag="pool_ps")
                for g in range(n_groups):
                    bufs = []
                    CH = 8                       # tiles per dma_gather (1024 idxs)
                    n_sub = math.ceil(GB * T2 / CH)
                    for h, src_ap in ((0, x_lo), (1, x_hi)):
                        buf = gp.tile([P, GB * T2, IN_DIM], f16,
                                      tag=f"gat{h}")
                        goff = (g * 2 + h) * gcols
                        for s in range(n_sub):
                            t0 = s * CH
                            t1 = min(GB * T2, t0 + CH)
                            ni = (t1 - t0) * P
                            nc.gpsimd.dma_gather(
                                buf[:, t0:t1, :], src_ap,
                                gidx[:, goff + t0 * 8:goff + t1 * 8],
                                ni, ni, IN_DIM)
                        bufs.append(buf)

                    for b in range(GB):
                        bg = g * GB + b
                        out1 = ps1.tile([P, IN_DIM], f32, space="PSUM",
                                        tag="out1")
                        for h in range(2):
                            for j in range(T2):
                                c = ((g * 2 + h) * GB + b) * T2 + j
                                stw = wp.tile([P, P], f16, tag="stw")
                                nc.vector.tensor_scalar(
                                    out=stw[:], in0=iota16[:],
                                    scalar1=dstc[:, c:c + 1],
                                    scalar2=wc[:, c:c + 1],
                                    op0=mybir.AluOpType.is_equal,
                                    op1=mybir.AluOpType.mult)
                                nc.tensor.matmul(
                                    out=out1[:], lhsT=stw[:],
                                    rhs=bufs[h][:, b * T2 + j, :],
                                    start=(h == 0 and j == 0),
                                    stop=(h == 1 and j == T2 - 1))

                        # h1T = relu(W1^T OUT1^T + b1); z2 = h1 W2; pool += C^T z2
                        o1s = wp.tile([P, IN_DIM], f32, tag="o1s")
                        nc.scalar.activation(out=o1s[:], in_=out1[:],
                                             func=AF.Copy)
                        o1t_ps = ps2.tile([IN_DIM, P], f32, space="PSUM",
                                          tag="o1t")
                        nc.tensor.transpose(out=o1t_ps[:], in_=o1s[:],
                                            identity=ident[:])
                        o1t = wp.tile([IN_DIM, P], f16, tag="o1t_sb")
                        nc.vector.tensor_copy(out=o1t[:], in_=o1t_ps[:])
                        h1t_ps = ps2.tile([HID_DIM, P], f32, space="PSUM",
                                          tag="h1t")
                        nc.tensor.matmul(out=h1t_ps[:], lhsT=w1_t[:],
                                         rhs=o1t[:], start=True, stop=True)
                        h1t = wp.tile([HID_DIM, P], f16, tag="h1t_sb")
                        nc.scalar.activation(out=h1t[:], in_=h1t_ps[:],
                                             func=AF.Relu, bias=b1_t[:, :1])
                        z2_ps = ps2.tile([P, OUT_DIM], f32, space="PSUM",
                                         tag="z2")
                        nc.tensor.matmul(out=z2_ps[:], lhsT=h1t[:],
                                         rhs=w2_t[:], start=True, stop=True)
                        z2s = wp.tile([P, OUT_DIM], f16, tag="z2_sb")
                        nc.scalar.activation(out=z2s[:], in_=z2_ps[:],
                                             func=AF.Copy)
                        nc.tensor.matmul(
                            out=pool_ps[:],
                            lhsT=ctc[:, bg * N_GRAPHS:(bg + 1) * N_GRAPHS],
                            rhs=z2s[:], start=(bg == 0),
                            stop=(bg == n_blocks - 1))

                pool_sb = wp.tile([N_GRAPHS, OUT_DIM], f32, tag="pool_sb")
                nc.scalar.activation(out=pool_sb[:], in_=pool_ps[:],
                                     func=AF.Copy)
                nc.sync.dma_start(out=pool_d[:], in_=pool_sb[:])

    nc.compile()
    return nc


def _make_in_maps(x, W1, W2, b1, prep):
    x16 = np.ascontiguousarray(x.astype(np.float16))
    b1_col = np.ascontiguousarray(b1.reshape(HID_DIM, 1).astype(np.float32))
    w1_16 = W1.astype(np.float16)
    w2_16 = W2.astype(np.float16)
    iota16 = np.tile(np.arange(P, dtype=np.float16)[None, :], (P, 1))
    ident = np.eye(P, dtype=np.float32)
    in_maps = []
    for c in range(N_CORES):
        in_maps.append({
            "x16": x16,
            "w1": w1_16,
            "w2": w2_16,
            "b1": b1_col,
            "iota16": iota16,
            "ident": ident,
            "gidx": np.ascontiguousarray(prep["gidx"][c]),
            "w_cols": np.ascontiguousarray(prep["w_cols"][c]),
            "dstl_cols": np.ascontiguousarray(prep["dstl_cols"][c]),
            "ct_cols": np.ascontiguousarray(prep["CT_cols"][c]),
        })
    return in_maps


# ---------------------------------------------------------------- entry point
def kernel(x, edge_index, batch, W1, b1, W2, b2):
    from concourse.bass_utils import run_bass_kernel_spmd

    x = np.asarray(x, dtype=np.float32)
    W1 = np.asarray(W1, dtype=np.float32)
    b1 = np.asarray(b1, dtype=np.float32)
    W2 = np.asarray(W2, dtype=np.float32)
    b2 = np.asarray(b2, dtype=np.float32)

    prep = _host_prep(x, edge_index, batch)
    key = (prep["T2"], prep["n_blocks"], x.shape[0])
    if key not in _PROGRAM_CACHE:
        _PROGRAM_CACHE[key] = _build_program(*key)
    nc = _PROGRAM_CACHE[key]

    in_maps = _make_in_maps(x, W1, W2, b1, prep)
    res = run_bass_kernel_spmd(nc, in_maps, list(range(N_CORES)))
    globals()["_LAST_RESULT"] = res

    total = np.zeros((N_GRAPHS, OUT_DIM), dtype=np.float64)
    for c in range(N_CORES):
        total += res.results[c]["pool_out"].astype(np.float64)

    counts = np.maximum(prep["graph_counts"], 1.0).astype(np.float32)
    out = (total.astype(np.float32) / counts[:, None]) + b2[None, :]
    return out.astype(np.float32)



# revision 52
# speedup vs baseline: 208.8443x; 208.8443x over previous
"""GCN classifier Trainium2 kernel (8-core SPMD, Bass/Tile).

Model (reference):
    h1 = relu(gcnconv(x, W1, b1));  h2 = gcnconv(h1, W2, b2);  out = mean-pool(h2, batch)

Distribution strategy (no cross-core communication):
  * Nodes sharded contiguously across 8 cores (6250 each); x replicated (fp16).
  * Layer-1 aggregation per dst shard. Edge rows are fetched with BULK
    dma_gather (mlp gpsimd library): one instruction gathers ~7k rows for a
    7-block group, split into lo/hi source halves because gather indices are
    int16. Per 128-edge tile, the selection matrix
    S_T[e,d] = w_e * (dst_e == d) (w_e = dinv[src]*dinv[dst]) is built in one
    fused DVE tensor_scalar; scatter = fp16 matmul accumulating into f32 PSUM.
  * h1^T and z2 = h1 @ W2 stay on-chip (PSUM/SBUF), never round-trip HBM.
  * Layer 2 + mean-pool collapse algebraically:
        pool_sums[g,f] = sum_e w_e * z2[src_e, f] * [batch[dst_e] == g]
                       = sum_n C[g,n] * z2[n,f]
    with C built host-side from indices/weights only -> dense matmuls, zero
    communication. Host sums 8 partials, divides by counts, adds b2.

Numerics: fp16 operand quantization (~5e-4) with exact f32 PSUM accumulation.
(True f32 matmuls on TRN2 PE are bf16-class 2-pass approximations anyway.)
"""

import math
import numpy as np

N_NODES = 50000
N_EDGES = 600000
N_GRAPHS = 64
IN_DIM = 128
HID_DIM = 128
OUT_DIM = 64
N_CORES = 8
P = 128
GB = 7              # blocks per gather group
HALF = N_NODES // 2
CH = 8              # tiles per dma_gather (1024 idxs; >8 crashes the exec unit)
N_QUEUES = 4        # SWDGE queues (ucode max 4); gather is desc-gen bound
USE_INDIRECT = False


# ---------------------------------------------------------------- host prep
def _host_prep(x, edge_index, batch):
    n = x.shape[0]
    half = n // 2
    shard = n // N_CORES                    # 6250
    n_blocks = math.ceil(shard / P)         # 49
    shard_pad = n_blocks * P                # 6272
    n_groups = math.ceil(n_blocks / GB)     # 7

    src = np.asarray(edge_index[0], dtype=np.int64)
    dst = np.asarray(edge_index[1], dtype=np.int64)
    batch = np.asarray(batch, dtype=np.int64)

    deg = np.bincount(dst, minlength=n).astype(np.float32) + np.float32(1.0)
    dinv = (np.float32(1.0) / np.sqrt(deg)).astype(np.float32)

    loops = np.arange(n, dtype=np.int64)
    SRC = np.concatenate([src, loops])
    DST = np.concatenate([dst, loops])
    W = (dinv[SRC] * dinv[DST]).astype(np.float32)
    E = SRC.shape[0]

    # ---- bucket edges by (core, block, src-half) of DST/SRC
    core_of = DST // shard
    blk_of = (DST % shard) // P
    dstl = (DST % shard) % P
    half_of = SRC // half

    n_buckets_per_core = n_blocks * 2
    bucket = (core_of * n_blocks + blk_of) * 2 + half_of
    order = np.argsort(bucket, kind="stable")
    bucket_s = bucket[order]
    counts = np.bincount(bucket_s, minlength=N_CORES * n_buckets_per_core)
    T2 = int(math.ceil(counts.max() / P))   # uniform tiles per (block, half)

    cum = np.zeros(N_CORES * n_buckets_per_core + 1, dtype=np.int64)
    np.cumsum(counts, out=cum[1:])
    pos = np.arange(E) - cum[bucket_s]

    # tile-column order: col(g, half, b_in_g, j) = ((g*2 + half)*GB + b)*T2 + j
    ntiles = n_blocks * 2 * T2              # columns per core
    e_core = core_of[order]
    e_blk = blk_of[order]
    e_half = half_of[order]
    e_g = e_blk // GB
    e_b = e_blk % GB
    col = ((e_g * 2 + e_half) * GB + e_b) * T2 + pos // P
    row = pos % P

    w_cols = np.zeros((N_CORES, P, ntiles), dtype=np.float32)
    dstl_cols = np.zeros((N_CORES, P, ntiles), dtype=np.float32)
    w_cols[e_core, row, col] = W[order]
    dstl_cols[e_core, row, col] = dstl[order].astype(np.float32)

    # gather indices: flat slot i = col*128 + row; idx layout [16, i//16] tiled
    # padding = -1: trailing invalid idxs are skipped by the gather when the
    # runtime count register (gcnt) is below the static num_idxs
    pad_skip_ok = (CH == T2)
    flat_idx = np.full((N_CORES, ntiles * P), -1 if pad_skip_ok else 0,
                       dtype=np.int16)
    slot = col * P + row
    flat_idx[e_core, slot] = (SRC[order] % half).astype(np.int16)
    nic = ntiles * P // 16                  # int16 idx columns per core
    gidx16 = flat_idx.reshape(N_CORES, nic, 16).transpose(0, 2, 1)  # [C,16,nic]
    gidx = np.ascontiguousarray(
        np.tile(gidx16, (1, 8, 1)))         # replicate to [C, 128, nic]

    # absolute int32 indices for indirect DMA gather; padding -> OOB (skipped)
    flat32 = np.full((N_CORES, ntiles * P), 2 ** 30, dtype=np.int32)
    flat32[e_core, slot] = SRC[order].astype(np.int32)
    idx32 = np.ascontiguousarray(
        flat32.reshape(N_CORES, ntiles, P).transpose(0, 2, 1))  # [C,128,ntiles]

    # per-gather valid-idx counts: gather (g, h, s) covers bucket (blk=g*GB+s,
    # half=h) exactly when CH == T2; order matches the kernel's gather loop.
    # Fallback (T2 != CH): full per-gather slot counts (no skipping).
    cnts = counts.reshape(N_CORES, n_blocks, 2)
    n_sub = math.ceil(GB * T2 / CH)
    gcnt = np.zeros((N_CORES, 1, n_groups * 2 * n_sub), dtype=np.int32)
    for g in range(n_groups):
        for h in range(2):
            for s in range(n_sub):
                gi = (g * 2 + h) * n_sub + s
                if pad_skip_ok:
                    gcnt[:, 0, gi] = cnts[:, g * GB + s, h]
                else:
                    t0 = s * CH
                    t1 = min(GB * T2, t0 + CH)
                    gcnt[:, 0, gi] = (t1 - t0) * P
    if pad_skip_ok:
        assert gcnt.min() >= 1, "empty gather bucket unsupported"

    # ---- layer-2 dense matrix C[g, n] = sum_{e: src=n} w_e * [batch[dst_e]=g]
    g_of = batch[DST]
    idx = ((SRC // shard) * N_GRAPHS + g_of) * shard + (SRC % shard)
    C = np.bincount(idx, weights=W.astype(np.float64),
                    minlength=N_CORES * N_GRAPHS * shard)
    C = C.reshape(N_CORES, N_GRAPHS, shard).astype(np.float32)

    Cp = np.zeros((N_CORES, N_GRAPHS, shard_pad), dtype=np.float32)
    Cp[:, :, :shard] = C
    CT_cols = Cp.reshape(N_CORES, N_GRAPHS, n_blocks, P).transpose(0, 3, 2, 1)
    CT_cols = np.ascontiguousarray(
        CT_cols.reshape(N_CORES, P, n_blocks * N_GRAPHS)).astype(np.float16)

    graph_counts = np.bincount(batch, minlength=N_GRAPHS).astype(np.float32)

    return dict(T2=T2, n_blocks=n_blocks, n_groups=n_groups, shard=shard,
                w_cols=w_cols, dstl_cols=dstl_cols, gidx=gidx, idx32=idx32,
                gcnt=gcnt, pad_skip_ok=pad_skip_ok,
                CT_cols=CT_cols, graph_counts=graph_counts)


# ---------------------------------------------------------------- bass program
_PROGRAM_CACHE = {}


def _build_program(T2, n_blocks, n_nodes, repeat=1, skip_gather=False,
                   skip_compute=False, skip_sbuild=False, skip_chain=False,
                   n_queues=N_QUEUES, use_indirect=False, pad_skip=True,
                   gat_bufs=3, stw_bufs=16):
    import concourse.bacc as bacc
    import concourse.tile as tile
    from concourse import mybir

    from concourse.bass import IndirectOffsetOnAxis

    f32, i32 = mybir.dt.float32, mybir.dt.int32
    f16, i16 = mybir.dt.float16, mybir.dt.int16
    AF = mybir.ActivationFunctionType

    half = n_nodes // 2
    n_groups = math.ceil(n_blocks / GB)
    ntiles = n_blocks * 2 * T2
    nic = ntiles * P // 16
    gidx_per_gather = GB * T2 * P           # num_idxs per dma_gather
    gcols = gidx_per_gather // 16           # int16 cols per gather slice

    nc = bacc.Bacc("TRN2", target_bir_lowering=False, debug=False,
                   num_devices=N_CORES, num_swdge_queues=n_queues)
    x16_d = nc.dram_tensor("x16", [n_nodes, IN_DIM], f16, kind="ExternalInput")
    w1_d = nc.dram_tensor("w1", [IN_DIM, HID_DIM], f16, kind="ExternalInput")
    w2_d = nc.dram_tensor("w2", [HID_DIM, OUT_DIM], f16, kind="ExternalInput")
    b1_d = nc.dram_tensor("b1", [HID_DIM, 1], f32, kind="ExternalInput")
    iota_d = nc.dram_tensor("iota16", [P, P], f16, kind="ExternalInput")
    ident_d = nc.dram_tensor("ident", [P, P], f32, kind="ExternalInput")
    gidx_d = nc.dram_tensor("gidx", [P, nic], i16, kind="ExternalInput")
    idx32_d = nc.dram_tensor("idx32", [P, ntiles], i32, kind="ExternalInput")
    n_gathers = n_groups * 2 * math.ceil(GB * T2 / CH)
    gcnt_d = nc.dram_tensor("gcnt", [1, n_gathers], i32, kind="ExternalInput")
    wc_d = nc.dram_tensor("w_cols", [P, ntiles], f32, kind="ExternalInput")
    dstc_d = nc.dram_tensor("dstl_cols", [P, ntiles], f32, kind="ExternalInput")
    ctc_d = nc.dram_tensor("ct_cols", [P, n_blocks * N_GRAPHS], f16,
                           kind="ExternalInput")
    pool_d = nc.dram_tensor("pool_out", [N_GRAPHS, OUT_DIM], f32,
                            kind="ExternalOutput")

    with tile.TileContext(nc) as tc:
        with (
            tc.tile_pool(name="const", bufs=1) as cp,
            tc.tile_pool(name="work", bufs=4) as wp,
            tc.tile_pool(name="stwp", bufs=stw_bufs) as sp,
            tc.tile_pool(name="gat", bufs=gat_bufs) as gp,
            tc.tile_pool(name="ps_o1t", bufs=2, space="PSUM") as pso,
            tc.tile_pool(name="ps_misc", bufs=2, space="PSUM") as ps2,
            tc.tile_pool(name="ps_pool", bufs=1, space="PSUM") as psp,
        ):
            iota16 = cp.tile([P, P], f16)
            nc.sync.dma_start(out=iota16[:], in_=iota_d[:])
            ident = cp.tile([P, P], f32)
            nc.sync.dma_start(out=ident[:], in_=ident_d[:])
            w1_t = cp.tile([IN_DIM, HID_DIM], f16)
            nc.sync.dma_start(out=w1_t[:], in_=w1_d[:])
            w2_t = cp.tile([HID_DIM, OUT_DIM], f16)
            nc.sync.dma_start(out=w2_t[:], in_=w2_d[:])
            b1_t = cp.tile([HID_DIM, 1], f32)
            nc.sync.dma_start(out=b1_t[:], in_=b1_d[:])
            gidx = cp.tile([P, nic], i16)
            nc.sync.dma_start(out=gidx[:], in_=gidx_d[:])
            idx32 = cp.tile([P, ntiles], i32)
            nc.sync.dma_start(out=idx32[:], in_=idx32_d[:])
            gcnt = cp.tile([1, n_gathers], i32)
            nc.sync.dma_start(out=gcnt[:], in_=gcnt_d[:])
            cnt_reg = nc.gpsimd.alloc_register("gcnt_reg")
            wc = cp.tile([P, ntiles], f32)
            nc.sync.dma_start(out=wc[:], in_=wc_d[:])
            dstc = cp.tile([P, ntiles], f32)
            nc.sync.dma_start(out=dstc[:], in_=dstc_d[:])
            ctc = cp.tile([P, n_blocks * N_GRAPHS], f16)
            nc.sync.dma_start(out=ctc[:], in_=ctc_d[:])

            x_lo = x16_d[0:half, :]
            x_hi = x16_d[half:n_nodes, :]

            if use_indirect or pad_skip:
                # zero the gather ring once: skipped padding slots must never
                # expose uninitialized SBUF (NaN x 0 = NaN in PSUM)
                for _r in range(gat_bufs):
                    for h in range(2):
                        zb = gp.tile([P, GB * T2, IN_DIM], f16, tag=f"gat{h}")
                        nc.vector.memset(zb[:], 0.0)

            stw_pre = []
            if skip_sbuild:
                for k in range(4):
                    t = cp.tile([P, P], f16, tag=f"stw_pre{k}")
                    nc.vector.tensor_scalar(
                        out=t[:], in0=iota16[:], scalar1=dstc[:, k:k + 1],
                        scalar2=wc[:, k:k + 1],
                        op0=mybir.AluOpType.is_equal,
                        op1=mybir.AluOpType.mult)
                    stw_pre.append(t)

            for _rep in range(repeat):
                if not skip_compute:
                    pool_ps = psp.tile([N_GRAPHS, OUT_DIM], f32, space="PSUM",
                                       tag="pool_ps")
                for g in range(n_groups):
                    bufs = []
                    n_sub = math.ceil(GB * T2 / CH)
                    for h, src_ap in ((0, x_lo), (1, x_hi)):
                        buf = gp.tile([P, GB * T2, IN_DIM], f16,
                                      tag=f"gat{h}")
                        goff = (g * 2 + h) * gcols
                        if skip_gather:
                            nc.vector.memset(buf[:, 0:1, :], 0.25)
                        elif use_indirect:
                            base = (g * 2 + h) * GB * T2
                            for t in range(GB * T2):
                                nc.gpsimd.indirect_dma_start(
                                    out=buf[:, t, :], out_offset=None,
                                    in_=x16_d[:, :],
                                    in_offset=IndirectOffsetOnAxis(
                                        ap=idx32[:, base + t:base + t + 1],
                                        axis=0),
                                    bounds_check=n_nodes - 1,
                                    oob_is_err=False)
                        else:
                            for s in range(n_sub):
                                t0 = s * CH
                                t1 = min(GB * T2, t0 + CH)
                                ni = (t1 - t0) * P
                                gi = (g * 2 + h) * n_sub + s
                                if pad_skip:
                                    nc.gpsimd.reg_load(
                                        cnt_reg, gcnt[0:1, gi:gi + 1])
                                    ni_reg = cnt_reg
                                else:
                                    ni_reg = ni
                                nc.gpsimd.dma_gather(
                                    buf[:, t0:t1, :], src_ap,
                                    gidx[:, goff + t0 * 8:goff + t1 * 8],
                                    ni, ni_reg, IN_DIM,
                                    queue_num=gi % n_queues)
                        bufs.append(buf)

                    for b in range(GB if not skip_compute else 0):
                        bg = g * GB + b
                        # o1t_ps[f, d] = sum_e Xg[e, f] * S[e, d]  (= out1^T)
                        o1t_ps = pso.tile([IN_DIM, P], f32, space="PSUM",
                                          tag="o1t")
                        for h in range(2):
                            for j in range(T2):
                                c = ((g * 2 + h) * GB + b) * T2 + j
                                if skip_sbuild:
                                    stw = stw_pre[(h * T2 + j) % 4]
                                else:
                                    stw = sp.tile([P, P], f16, tag="stw")
                                    nc.vector.tensor_scalar(
                                        out=stw[:], in0=iota16[:],
                                        scalar1=dstc[:, c:c + 1],
                                        scalar2=wc[:, c:c + 1],
                                        op0=mybir.AluOpType.is_equal,
                                        op1=mybir.AluOpType.mult)
                                nc.tensor.matmul(
                                    out=o1t_ps[:],
                                    lhsT=bufs[h][:, b * T2 + j, :],
                                    rhs=stw[:],
                                    start=(h == 0 and j == 0),
                                    stop=(h == 1 and j == T2 - 1))

                        if skip_chain:
                            continue
                        # h1T = relu(W1^T o1t + b1); z2 = h1 W2; pool += C^T z2
                        o1t = wp.tile([IN_DIM, P], f16, tag="o1t_sb")
                        nc.scalar.activation(out=o1t[:], in_=o1t_ps[:],
                                             func=AF.Copy)
                        h1t_ps = ps2.tile([HID_DIM, P], f32, space="PSUM",
                                          tag="h1t")
                        nc.tensor.matmul(out=h1t_ps[:], lhsT=w1_t[:],
                                         rhs=o1t[:], start=True, stop=True)
                        h1t = wp.tile([HID_DIM, P], f16, tag="h1t_sb")
                        nc.scalar.activation(out=h1t[:], in_=h1t_ps[:],
                                             func=AF.Relu, bias=b1_t[:, :1])
                        z2_ps = ps2.tile([P, OUT_DIM], f32, space="PSUM",
                                         tag="z2")
                        nc.tensor.matmul(out=z2_ps[:], lhsT=h1t[:],
                                         rhs=w2_t[:], start=True, stop=True)
                        z2s = wp.tile([P, OUT_DIM], f16, tag="z2_sb")
                        nc.scalar.activation(out=z2s[:], in_=z2_ps[:],
                                             func=AF.Copy)
                        nc.tensor.matmul(
                            out=pool_ps[:],
                            lhsT=ctc[:, bg * N_GRAPHS:(bg + 1) * N_GRAPHS],
                            rhs=z2s[:], start=(bg == 0),
                            stop=(bg == n_blocks - 1))

                pool_sb = wp.tile([N_GRAPHS, OUT_DIM], f32, tag="pool_sb")
                if not (skip_compute or skip_chain):
                    nc.scalar.activation(out=pool_sb[:], in_=pool_ps[:],
                                         func=AF.Copy)
                else:
                    nc.scalar.activation(out=pool_sb[:],
                                         in_=ident[:N_GRAPHS, :OUT_DIM],
                                         func=AF.Copy)
                nc.sync.dma_start(out=pool_d[:], in_=pool_sb[:])

    nc.compile()
    return nc


def _make_in_maps(x, W1, W2, b1, prep):
    x16 = np.ascontiguousarray(x.astype(np.float16))
    b1_col = np.ascontiguousarray(b1.reshape(HID_DIM, 1).astype(np.float32))
    w1_16 = W1.astype(np.float16)
    w2_16 = W2.astype(np.float16)
    iota16 = np.tile(np.arange(P, dtype=np.float16)[None, :], (P, 1))
    ident = np.eye(P, dtype=np.float32)
    in_maps = []
    for c in range(N_CORES):
        in_maps.append({
            "x16": x16,
            "w1": w1_16,
            "w2": w2_16,
            "b1": b1_col,
            "iota16": iota16,
            "ident": ident,
            "gidx": np.ascontiguousarray(prep["gidx"][c]),
            "idx32": np.ascontiguousarray(prep["idx32"][c]),
            "gcnt": np.ascontiguousarray(prep["gcnt"][c]),
            "w_cols": np.ascontiguousarray(prep["w_cols"][c]),
            "dstl_cols": np.ascontiguousarray(prep["dstl_cols"][c]),
            "ct_cols": np.ascontiguousarray(prep["CT_cols"][c]),
        })
    return in_maps


# ---------------------------------------------------------------- entry point
def kernel(x, edge_index, batch, W1, b1, W2, b2):
    from concourse.bass_utils import run_bass_kernel_spmd

    x = np.asarray(x, dtype=np.float32)
    W1 = np.asarray(W1, dtype=np.float32)
    b1 = np.asarray(b1, dtype=np.float32)
    W2 = np.asarray(W2, dtype=np.float32)
    b2 = np.asarray(b2, dtype=np.float32)

    prep = _host_prep(x, edge_index, batch)
    key = (prep["T2"], prep["n_blocks"], x.shape[0])
    if key not in _PROGRAM_CACHE:
        _PROGRAM_CACHE[key] = _build_program(*key, use_indirect=USE_INDIRECT)
    nc = _PROGRAM_CACHE[key]

    in_maps = _make_in_maps(x, W1, W2, b1, prep)
    res = run_bass_kernel_spmd(nc, in_maps, list(range(N_CORES)))
    globals()["_LAST_RESULT"] = res

    total = np.zeros((N_GRAPHS, OUT_DIM), dtype=np.float64)
    for c in range(N_CORES):
        total += res.results[c]["pool_out"].astype(np.float64)

    counts = np.maximum(prep["graph_counts"], 1.0).astype(np.float32)
    out = (total.astype(np.float32) / counts[:, None]) + b2[None, :]
    return out.astype(np.float32)

